# revision 40
# baseline (speedup 1.0000x reference)
"""Local-sparse-attention (inverted band mask) Bass kernel for 8 TRN2 cores.

Sharding: one head per core (H=8). Each core computes qkv projection for its
head, dense attention (band-EXCLUDED mask) over both batches, and a partial
output projection. Host sums the 8 partials and adds bias.

Device dataflow (per core, head h):
  xT [512, 4096]  (D on partitions)  --matmul-->  qT,kT [64, 4096], V [S,64]
  ST[keys,q] = kT.T-as-lhsT @ qT     (K=Dh=64, float32r; exp(temp) folded
  into Wq on host)
  PT = exp(ST)  (ACT, PSUM->SBUF bf16), band mask applied multiplicatively
  avT[dh,q] (+ones row -> key sums) = V|1 as lhsT @ PT   (K=128 keys, bf16)
  normalize: sums row replicated down 64 partitions via K=1 matmul,
  reciprocal_approx_fast, fused into the av PSUM->SBUF copy; out-proj
  result DMAs straight from PSUM.

Engine budget per core: PE ~217k cy, ACT 64 exp instrs (~70us), DVE
(masks+kT+recip+avs), Pool (qT+V+srow copies). st PSUM double-buffered;
batch-1 projection interleaved into batch-0 attention to fill PE bubbles.
"""

import sys

if "/opt/trn_rl_repo" not in sys.path:
    sys.path.insert(0, "/opt/trn_rl_repo")

import numpy as np

HEADS = 8
DH = 64
B, S, D = 2, 2048, 512
SF = B * S  # 4096 flattened rows
WINDOW = 8


def _sn_scale(W, u, sigma):
    """Scalar multiplier sigma/sigma_w of the spectral-norm reparam (fp32)."""
    W = W.astype(np.float32)
    u = u.astype(np.float32)
    v = W @ u
    v = v / np.linalg.norm(v)
    u2 = W.T @ v
    u2 = u2 / np.linalg.norm(u2)
    sigma_w = v @ (W @ u2)
    return np.float32(sigma / sigma_w)


def _masks():
    import ml_dtypes

    jl = np.arange(128)[:, None]  # keys (partitions)
    il = np.arange(128)[None, :]  # queries (free)
    mdiag = np.where((jl >= il - (WINDOW - 1)) & (jl <= il), 0.0, 1.0)
    msub = np.where(jl >= il + 128 - (WINDOW - 1), 0.0, 1.0)
    return (
        mdiag.astype(ml_dtypes.bfloat16),
        msub.astype(ml_dtypes.bfloat16),
    )


def _build(exp_temp: float, null=False, reps=1):
    import concourse.bass as bass
    import concourse.mybir as mybir
    import concourse.tile as tile
    from concourse import bacc

    f32 = mybir.dt.float32
    f32r = mybir.dt.float32r
    bf16 = mybir.dt.bfloat16
    nc = bacc.Bacc()

    xT_d = nc.dram_tensor("xT", [D, SF], bf16, kind="ExternalInput").ap()
    wqT_d = nc.dram_tensor("wqT", [D, 128], bf16, kind="ExternalInput").ap()
    wvT_d = nc.dram_tensor("wvT", [D, DH], bf16, kind="ExternalInput").ap()
    woT_d = nc.dram_tensor("woT", [DH, D], bf16, kind="ExternalInput").ap()
    mdiag_d = nc.dram_tensor("mdiag", [128, 128], bf16, kind="ExternalInput").ap()
    msub_d = nc.dram_tensor("msub", [128, 128], bf16, kind="ExternalInput").ap()
    onesr_d = nc.dram_tensor("onesr", [128, 1], bf16, kind="ExternalInput").ap()
    out_d = nc.dram_tensor("part", [SF, D], bf16, kind="ExternalOutput").ap()

    Exp = mybir.ActivationFunctionType.Exp
    mult = mybir.AluOpType.mult

    if null:
        with tile.TileContext(nc) as tc:
            with tc.tile_pool(name="nul", bufs=1) as npool:
                t = npool.tile([128, 128], f32)
                nc.vector.memset(t, 0.0)
                nc.sync.dma_start(out_d[0:128, 0:128], t)
        return nc

    with tile.TileContext(nc) as tc:
        with (
            tc.tile_pool(name="const", bufs=1) as cpool,
            tc.tile_pool(name="pt", bufs=2) as ptpool,
            tc.tile_pool(name="sb", bufs=3) as sbpool,
            tc.tile_pool(name="stp", bufs=2, space="PSUM") as stpool,
            tc.tile_pool(name="avp", bufs=1, space="PSUM") as avpool,
            tc.tile_pool(name="mmp", bufs=3, space="PSUM") as mmpool,
        ):
            # ---- constants / weights ----
            wq = cpool.tile([128, 4, 128], bf16)
            wv = cpool.tile([128, 4, DH], bf16)
            nc.sync.dma_start(wq, wqT_d.rearrange("(c p) m -> p c m", p=128))
            nc.sync.dma_start(wv, wvT_d.rearrange("(c p) m -> p c m", p=128))
            wo = cpool.tile([DH, D], bf16)
            nc.sync.dma_start(wo, woT_d)
            # combined band masks: [:, 0, :] = diag chunk, [:, 1, :] = subdiag
            mm2 = cpool.tile([128, 2, 128], bf16)
            nc.sync.dma_start(mm2[:, 0, :], mdiag_d)
            nc.sync.dma_start(mm2[:, 1, :], msub_d)
            onesr = cpool.tile([128, 1], bf16)
            nc.sync.dma_start(onesr, onesr_d)

            qT = cpool.tile([DH, SF], bf16)
            kT = cpool.tile([DH, SF], bf16)
            V = cpool.tile([128, 32, 128], bf16)  # [keys, s-chunk, dh|1]
            # whole-tile memset (contiguous); the per-chunk V copies overwrite
            # cols 0:DH, leaving col DH == 1.0 as the key-count ones column
            nc.vector.memset(V, 1.0)
            xTs = cpool.tile([128, 4, SF], bf16)

            def emit_qk_block(sc):
                """qT/kT for a 512-wide column block."""
                sl = slice(sc * 512, (sc + 1) * 512)
                psq = mmpool.tile([128, 512], f32, tag="mm")
                for c in range(4):
                    nc.tensor.matmul(
                        psq, wq[:, c, :], xTs[:, c, sl],
                        start=(c == 0), stop=(c == 3),
                    )
                nc.vector.tensor_copy(qT[:, sl], psq[0:DH, :])
                nc.vector.tensor_copy(kT[:, sl], psq[DH:128, :])

            def emit_v_block(sc):
                """V rows [sc*128, (sc+1)*128) in natural [keys, dh] layout."""
                psv = mmpool.tile([128, DH], f32, tag="mm")
                for c in range(4):
                    nc.tensor.matmul(
                        psv, xTs[:, c, sc * 128 : (sc + 1) * 128], wv[:, c, :],
                        start=(c == 0), stop=(c == 3),
                    )
                nc.vector.tensor_copy(V[:, sc, 0:DH], psv)

            def emit_attn_block(b, qb):
                """Attention + out-proj for 512 queries, software-pipelined:
                score matmuls of group g are emitted before the av matmuls of
                group g-1 so PE streams while ACT computes exp."""
                qoff = b * S + qb * 512
                pt = ptpool.tile([128, 16, 512], bf16, tag="pt")
                av = avpool.tile([128, 512], f32, tag="av")

                def emit_scores(g):
                    st = stpool.tile([128, 2, 512], f32, tag="st")
                    for j in range(2):
                        kc = g * 2 + j
                        nc.tensor.matmul(
                            st[:, j, :],
                            kT[:, b * S + kc * 128 : b * S + (kc + 1) * 128],
                            qT[:, qoff : qoff + 512],
                            start=True, stop=True,
                        )
                    nc.scalar.activation(pt[:, g * 2 : g * 2 + 2, :], st, Exp)
                    # band mask (multiplicative zero): for key chunk kc the
                    # diag mask hits qsub dq=kc-4qb and the subdiag mask hits
                    # dq+1; adjacent, so one op covers both when in range
                    for j in range(2):
                        kc = g * 2 + j
                        dq = kc - 4 * qb
                        if 0 <= dq <= 2:
                            sl2 = slice(dq * 128, (dq + 2) * 128)
                            nc.gpsimd.tensor_tensor(
                                pt[:, kc, sl2], pt[:, kc, sl2], mm2, mult
                            )
                        elif dq == 3:
                            sl2 = slice(3 * 128, 4 * 128)
                            nc.gpsimd.tensor_tensor(
                                pt[:, kc, sl2], pt[:, kc, sl2], mm2[:, 0, :], mult
                            )
                        elif dq == -1:
                            sl2 = slice(0, 128)
                            nc.gpsimd.tensor_tensor(
                                pt[:, kc, sl2], pt[:, kc, sl2], mm2[:, 1, :], mult
                            )

                def emit_av(g):
                    for j in range(2):
                        kc = g * 2 + j
                        nc.tensor.matmul(
                            av,
                            V[:, b * 16 + kc, :],
                            pt[:, kc, :],
                            start=(kc == 0), stop=(kc == 15),
                        )

                emit_scores(0)
                for g in range(1, 8):
                    emit_scores(g)
                    emit_av(g - 1)
                emit_av(7)

                # tail: one PSUM->SBUF copy of av|sums, transpose the sums
                # row to [q,1] via K=1 matmuls, per-partition reciprocal,
                # then out-proj with the 1/sum scale fused into the
                # PSUM->SBUF copy of each out tile
                avs = sbpool.tile([DH + 1, 512], bf16, tag="avs")
                nc.vector.tensor_copy(avs, av[0 : DH + 1, :])
                sums = mmpool.tile([128, 512], f32, tag="mm")
                for qsub in range(4):
                    nc.tensor.matmul(
                        sums[:, qsub : qsub + 1],
                        avs[DH : DH + 1, qsub * 128 : (qsub + 1) * 128],
                        onesr[DH : DH + 1, :],
                        start=True, stop=True,
                    )
                recips = sbpool.tile([128, 4], f32, tag="recips")
                nc.vector.reciprocal(recips, sums[:, 0:4])
                for qsub in range(4):
                    op = mmpool.tile([128, 512], f32, tag="mm")
                    nc.tensor.matmul(
                        op, avs[0:DH, qsub * 128 : (qsub + 1) * 128], wo,
                        start=True, stop=True,
                    )
                    ot = sbpool.tile([128, 512], bf16, tag="ot")
                    nc.vector.tensor_scalar(
                        ot, op, recips[:, qsub : qsub + 1], None, mult
                    )
                    r0 = qoff + qsub * 128
                    nc.sync.dma_start(out_d[r0 : r0 + 128, :], ot)

            for rep_i in range(reps):
                # xT DMA, batch 0 halves first so b0 projection starts early
                for bb in range(B):
                    for c in range(4):
                        nc.sync.dma_start(
                            xTs[:, c, bb * S : (bb + 1) * S],
                            xT_d[c * 128 : (c + 1) * 128, bb * S : (bb + 1) * S],
                        )
                # batch-0 projection
                for sc in range(4):
                    emit_qk_block(sc)
                for sc in range(16):
                    emit_v_block(sc)
                # batch-0 attention with batch-1 projection interleaved
                for qb in range(4):
                    emit_qk_block(4 + qb)
                    for sv in range(4):
                        emit_v_block(16 + qb * 4 + sv)
                    emit_attn_block(0, qb)
                # batch-1 attention
                for qb in range(4):
                    emit_attn_block(1, qb)
    return nc


def _pad128(a):
    out = np.zeros((a.shape[0], 128), dtype=a.dtype)
    out[:, : a.shape[1]] = a
    return out


def prepare(inputs, null=False, reps=1):
    """Host-side packing: returns (finalized nc, per-core in_maps)."""
    import ml_dtypes

    x = inputs["x"].astype(np.float32)
    W_qkv = inputs["W_qkv"].astype(np.float32)
    W_out = inputs["W_out"].astype(np.float32)
    s_qkv = _sn_scale(W_qkv, inputs["u_qkv"], inputs["sigma_qkv"][0])
    s_out = _sn_scale(W_out, inputs["u_out"], inputs["sigma_out"][0])
    Wq_eff = W_qkv * s_qkv  # [1536, 512]
    Wo_eff = W_out * s_out  # [512, 512]
    exp_temp = float(np.exp(np.float32(inputs["temperature"])))

    xT = np.ascontiguousarray(x.reshape(SF, D).T).astype(ml_dtypes.bfloat16)
    mdiag, msub = _masks()
    onesr = np.ones((128, 1), dtype=ml_dtypes.bfloat16)

    nc = _build(exp_temp, null=null, reps=reps)
    nc.finalize()  # Bacc: split multi-waits, alloc registers (pjrt path skips it)

    inner = HEADS * DH
    in_maps = []
    for h in range(HEADS):
        hs = slice(h * DH, (h + 1) * DH)
        in_maps.append({
            "xT": xT,
            # exp(temperature) score scale folded into Wq
            "wqT": np.concatenate([
                (np.ascontiguousarray(Wq_eff[hs, :].T) * np.float32(exp_temp)).astype(ml_dtypes.bfloat16),
                np.ascontiguousarray(Wq_eff[inner + h * DH : inner + (h + 1) * DH, :].T).astype(ml_dtypes.bfloat16),
            ], axis=1),
            "wvT": np.ascontiguousarray(Wq_eff[2 * inner + h * DH : 2 * inner + (h + 1) * DH, :].T).astype(ml_dtypes.bfloat16),
            "woT": np.ascontiguousarray(Wo_eff[:, hs].T).astype(ml_dtypes.bfloat16),
            "mdiag": mdiag,
            "msub": msub,
            "onesr": onesr,
        })
    return nc, in_maps


def finish(inputs, results) -> np.ndarray:
    """Host-side unshard: sum per-head partials, add bias."""
    b_out = inputs["b_out"].astype(np.float32)
    acc = np.zeros((SF, D), dtype=np.float32)
    for r in results:
        acc += r["part"].astype(np.float32)
    acc += b_out[None, :]
    return acc.reshape(B, S, D)


def kernel(**inputs) -> np.ndarray:
    from concourse.bass_utils import run_bass_kernel_spmd

    nc, in_maps = prepare(inputs)
    res = run_bass_kernel_spmd(nc, in_maps, core_ids=list(range(HEADS)))
    return finish(inputs, res.results)


# revision 41
# speedup vs baseline: 1.2682x; 1.2682x over previous
"""Local-sparse-attention (inverted band mask) Bass kernel for 8 TRN2 cores.

Sharding: one head per core (H=8). Each core computes qkv projection for its
head, dense attention (band-EXCLUDED mask) over both batches, and a partial
output projection. Host sums the 8 partials and adds bias.

Device dataflow (per core, head h):
  xT [512, 4096]  (D on partitions)  --matmul-->  qT,kT [64, 4096], V [S,64]
  ST[keys,q] = kT.T-as-lhsT @ qT     (K=Dh=64, float32r; exp(temp) folded
  into Wq on host)
  PT = exp(ST)  (ACT, PSUM->SBUF bf16), band mask applied multiplicatively
  avT[dh,q] (+ones row -> key sums) = V|1 as lhsT @ PT   (K=128 keys, bf16)
  normalize: sums row replicated down 64 partitions via K=1 matmul,
  reciprocal_approx_fast, fused into the av PSUM->SBUF copy; out-proj
  result DMAs straight from PSUM.

Engine budget per core: PE ~217k cy, ACT 64 exp instrs (~70us), DVE
(masks+kT+recip+avs), Pool (qT+V+srow copies). st PSUM double-buffered;
batch-1 projection interleaved into batch-0 attention to fill PE bubbles.
"""

import sys

if "/opt/trn_rl_repo" not in sys.path:
    sys.path.insert(0, "/opt/trn_rl_repo")

import numpy as np

HEADS = 8
DH = 64
B, S, D = 2, 2048, 512
SF = B * S  # 4096 flattened rows
WINDOW = 8


def _sn_scale(W, u, sigma):
    """Scalar multiplier sigma/sigma_w of the spectral-norm reparam (fp32)."""
    W = W.astype(np.float32)
    u = u.astype(np.float32)
    v = W @ u
    v = v / np.linalg.norm(v)
    u2 = W.T @ v
    u2 = u2 / np.linalg.norm(u2)
    sigma_w = v @ (W @ u2)
    return np.float32(sigma / sigma_w)


def _masks():
    import ml_dtypes

    jl = np.arange(128)[:, None]  # keys (partitions)
    il = np.arange(128)[None, :]  # queries (free)
    mdiag = np.where((jl >= il - (WINDOW - 1)) & (jl <= il), 0.0, 1.0)
    msub = np.where(jl >= il + 128 - (WINDOW - 1), 0.0, 1.0)
    return (
        mdiag.astype(ml_dtypes.bfloat16),
        msub.astype(ml_dtypes.bfloat16),
    )


def _build(exp_temp: float, null=False, reps=1):
    import concourse.bass as bass
    import concourse.mybir as mybir
    import concourse.tile as tile
    from concourse import bacc

    f32 = mybir.dt.float32
    f32r = mybir.dt.float32r
    bf16 = mybir.dt.bfloat16
    nc = bacc.Bacc()

    xT_d = nc.dram_tensor("xT", [D, SF], bf16, kind="ExternalInput").ap()
    wqT_d = nc.dram_tensor("wqT", [D, 128], bf16, kind="ExternalInput").ap()
    wvT_d = nc.dram_tensor("wvT", [D, DH], bf16, kind="ExternalInput").ap()
    woT_d = nc.dram_tensor("woT", [DH, D], bf16, kind="ExternalInput").ap()
    mdiag_d = nc.dram_tensor("mdiag", [128, 128], bf16, kind="ExternalInput").ap()
    msub_d = nc.dram_tensor("msub", [128, 128], bf16, kind="ExternalInput").ap()
    onesr_d = nc.dram_tensor("onesr", [128, 1], bf16, kind="ExternalInput").ap()
    out_d = nc.dram_tensor("part", [SF, D], bf16, kind="ExternalOutput").ap()

    Exp = mybir.ActivationFunctionType.Exp
    mult = mybir.AluOpType.mult

    if null:
        with tile.TileContext(nc) as tc:
            with tc.tile_pool(name="nul", bufs=1) as npool:
                t = npool.tile([128, 128], f32)
                nc.vector.memset(t, 0.0)
                nc.sync.dma_start(out_d[0:128, 0:128], t)
        return nc

    with tile.TileContext(nc) as tc:
        with (
            tc.tile_pool(name="const", bufs=1) as cpool,
            tc.tile_pool(name="pt", bufs=2) as ptpool,
            tc.tile_pool(name="sb", bufs=3) as sbpool,
            tc.tile_pool(name="stp", bufs=2, space="PSUM") as stpool,
            tc.tile_pool(name="avp", bufs=2, space="PSUM") as avpool,
            tc.tile_pool(name="mmp", bufs=2, space="PSUM") as mmpool,
        ):
            # ---- constants / weights ----
            wq = cpool.tile([128, 4, 128], bf16)
            wv = cpool.tile([128, 4, DH], bf16)
            nc.sync.dma_start(wq, wqT_d.rearrange("(c p) m -> p c m", p=128))
            nc.sync.dma_start(wv, wvT_d.rearrange("(c p) m -> p c m", p=128))
            wo = cpool.tile([DH, D], bf16)
            nc.sync.dma_start(wo, woT_d)
            # combined band masks: [:, 0, :] = diag chunk, [:, 1, :] = subdiag
            mm2 = cpool.tile([128, 2, 128], bf16)
            nc.sync.dma_start(mm2[:, 0, :], mdiag_d)
            nc.sync.dma_start(mm2[:, 1, :], msub_d)
            onesr = cpool.tile([128, 1], bf16)
            nc.sync.dma_start(onesr, onesr_d)

            qT = cpool.tile([DH, SF], bf16)
            kT = cpool.tile([DH, SF], bf16)
            V = cpool.tile([128, 32, 128], bf16)  # [keys, s-chunk, dh|1]
            # whole-tile memset (contiguous); the per-chunk V copies overwrite
            # cols 0:DH, leaving col DH == 1.0 as the key-count ones column
            nc.vector.memset(V, 1.0)
            xTs = cpool.tile([128, 4, SF], bf16)

            def emit_qk_block(sc):
                """qT/kT for a 512-wide column block."""
                sl = slice(sc * 512, (sc + 1) * 512)
                psq = mmpool.tile([128, 512], f32, tag="mm")
                for c in range(4):
                    nc.tensor.matmul(
                        psq, wq[:, c, :], xTs[:, c, sl],
                        start=(c == 0), stop=(c == 3),
                    )
                nc.vector.tensor_copy(qT[:, sl], psq[0:DH, :])
                nc.vector.tensor_copy(kT[:, sl], psq[DH:128, :])

            def emit_v_block(sc):
                """V rows [sc*128, (sc+1)*128) in natural [keys, dh] layout."""
                psv = mmpool.tile([128, DH], f32, tag="mm")
                for c in range(4):
                    nc.tensor.matmul(
                        psv, xTs[:, c, sc * 128 : (sc + 1) * 128], wv[:, c, :],
                        start=(c == 0), stop=(c == 3),
                    )
                nc.vector.tensor_copy(V[:, sc, 0:DH], psv)

            def emit_attn_block(b, qb):
                """Attention + out-proj for 512 queries, software-pipelined:
                score matmuls of group g are emitted before the av matmuls of
                group g-1 so PE streams while ACT computes exp."""
                qoff = b * S + qb * 512
                pt = ptpool.tile([128, 16, 512], bf16, tag="pt")
                av = avpool.tile([128, 512], f32, tag="av")

                def emit_scores(g):
                    st = stpool.tile([128, 2, 512], f32, tag="st")
                    for j in range(2):
                        kc = g * 2 + j
                        nc.tensor.matmul(
                            st[:, j, :],
                            kT[:, b * S + kc * 128 : b * S + (kc + 1) * 128],
                            qT[:, qoff : qoff + 512],
                            start=True, stop=True,
                        )
                    nc.scalar.activation(pt[:, g * 2 : g * 2 + 2, :], st, Exp)
                    # band mask (multiplicative zero): for key chunk kc the
                    # diag mask hits qsub dq=kc-4qb and the subdiag mask hits
                    # dq+1; adjacent, so one op covers both when in range
                    for j in range(2):
                        kc = g * 2 + j
                        dq = kc - 4 * qb
                        if 0 <= dq <= 2:
                            sl2 = slice(dq * 128, (dq + 2) * 128)
                            nc.gpsimd.tensor_tensor(
                                pt[:, kc, sl2], pt[:, kc, sl2], mm2, mult
                            )
                        elif dq == 3:
                            sl2 = slice(3 * 128, 4 * 128)
                            nc.gpsimd.tensor_tensor(
                                pt[:, kc, sl2], pt[:, kc, sl2], mm2[:, 0, :], mult
                            )
                        elif dq == -1:
                            sl2 = slice(0, 128)
                            nc.gpsimd.tensor_tensor(
                                pt[:, kc, sl2], pt[:, kc, sl2], mm2[:, 1, :], mult
                            )

                def emit_av(g):
                    for j in range(2):
                        kc = g * 2 + j
                        nc.tensor.matmul(
                            av,
                            V[:, b * 16 + kc, :],
                            pt[:, kc, :],
                            start=(kc == 0), stop=(kc == 15),
                        )

                emit_scores(0)
                for g in range(1, 8):
                    emit_scores(g)
                    emit_av(g - 1)
                emit_av(7)

                # tail: one PSUM->SBUF copy of av|sums, transpose the sums
                # row to [q,1] via K=1 matmuls, per-partition reciprocal,
                # then out-proj with the 1/sum scale fused into the
                # PSUM->SBUF copy of each out tile
                avs = sbpool.tile([DH + 1, 512], bf16, tag="avs")
                nc.vector.tensor_copy(avs, av[0 : DH + 1, :])
                sums = mmpool.tile([128, 512], f32, tag="mm")
                for qsub in range(4):
                    nc.tensor.matmul(
                        sums[:, qsub : qsub + 1],
                        avs[DH : DH + 1, qsub * 128 : (qsub + 1) * 128],
                        onesr[DH : DH + 1, :],
                        start=True, stop=True,
                    )
                recips = sbpool.tile([128, 4], f32, tag="recips")
                nc.vector.reciprocal(recips, sums[:, 0:4])
                for qsub in range(4):
                    op = mmpool.tile([128, 512], f32, tag="mm")
                    nc.tensor.matmul(
                        op, avs[0:DH, qsub * 128 : (qsub + 1) * 128], wo,
                        start=True, stop=True,
                    )
                    ot = sbpool.tile([128, 512], bf16, tag="ot")
                    nc.vector.tensor_scalar(
                        ot, op, recips[:, qsub : qsub + 1], None, mult
                    )
                    r0 = qoff + qsub * 128
                    nc.sync.dma_start(out_d[r0 : r0 + 128, :], ot)

            for rep_i in range(reps):
                # xT DMA, batch 0 halves first so b0 projection starts early
                for bb in range(B):
                    for c in range(4):
                        nc.sync.dma_start(
                            xTs[:, c, bb * S : (bb + 1) * S],
                            xT_d[c * 128 : (c + 1) * 128, bb * S : (bb + 1) * S],
                        )
                # batch-0 projection
                for sc in range(4):
                    emit_qk_block(sc)
                for sc in range(16):
                    emit_v_block(sc)
                # batch-0 attention with batch-1 projection interleaved
                for qb in range(4):
                    emit_qk_block(4 + qb)
                    for sv in range(4):
                        emit_v_block(16 + qb * 4 + sv)
                    emit_attn_block(0, qb)
                # batch-1 attention
                for qb in range(4):
                    emit_attn_block(1, qb)
    return nc


def _pad128(a):
    out = np.zeros((a.shape[0], 128), dtype=a.dtype)
    out[:, : a.shape[1]] = a
    return out


def prepare(inputs, null=False, reps=1):
    """Host-side packing: returns (finalized nc, per-core in_maps)."""
    import ml_dtypes

    x = inputs["x"].astype(np.float32)
    W_qkv = inputs["W_qkv"].astype(np.float32)
    W_out = inputs["W_out"].astype(np.float32)
    s_qkv = _sn_scale(W_qkv, inputs["u_qkv"], inputs["sigma_qkv"][0])
    s_out = _sn_scale(W_out, inputs["u_out"], inputs["sigma_out"][0])
    Wq_eff = W_qkv * s_qkv  # [1536, 512]
    Wo_eff = W_out * s_out  # [512, 512]
    exp_temp = float(np.exp(np.float32(inputs["temperature"])))

    xT = np.ascontiguousarray(x.reshape(SF, D).T).astype(ml_dtypes.bfloat16)
    mdiag, msub = _masks()
    onesr = np.ones((128, 1), dtype=ml_dtypes.bfloat16)

    nc = _build(exp_temp, null=null, reps=reps)
    nc.finalize()  # Bacc: split multi-waits, alloc registers (pjrt path skips it)

    inner = HEADS * DH
    in_maps = []
    for h in range(HEADS):
        hs = slice(h * DH, (h + 1) * DH)
        in_maps.append({
            "xT": xT,
            # exp(temperature) score scale folded into Wq
            "wqT": np.concatenate([
                (np.ascontiguousarray(Wq_eff[hs, :].T) * np.float32(exp_temp)).astype(ml_dtypes.bfloat16),
                np.ascontiguousarray(Wq_eff[inner + h * DH : inner + (h + 1) * DH, :].T).astype(ml_dtypes.bfloat16),
            ], axis=1),
            "wvT": np.ascontiguousarray(Wq_eff[2 * inner + h * DH : 2 * inner + (h + 1) * DH, :].T).astype(ml_dtypes.bfloat16),
            "woT": np.ascontiguousarray(Wo_eff[:, hs].T).astype(ml_dtypes.bfloat16),
            "mdiag": mdiag,
            "msub": msub,
            "onesr": onesr,
        })
    return nc, in_maps


def finish(inputs, results) -> np.ndarray:
    """Host-side unshard: sum per-head partials, add bias."""
    b_out = inputs["b_out"].astype(np.float32)
    acc = np.zeros((SF, D), dtype=np.float32)
    for r in results:
        acc += r["part"].astype(np.float32)
    acc += b_out[None, :]
    return acc.reshape(B, S, D)


def kernel(**inputs) -> np.ndarray:
    from concourse.bass_utils import run_bass_kernel_spmd

    nc, in_maps = prepare(inputs)
    res = run_bass_kernel_spmd(nc, in_maps, core_ids=list(range(HEADS)))
    return finish(inputs, res.results)
